# revision 30
# baseline (speedup 1.0000x reference)
"""Self-contained Trainium2 (Bass/Tile) kernel: single-head causal attention.

Problem: embeddings [4,4096,1024] f32; Wq/Wk/Wv [1024,1024] f32 (torch Linear
layout [out,in]).  out = softmax(causal(QK^T)/sqrt(D)) @ V, computed per batch.

Distribution: 8 NeuronCores, one SPMD program.  Core c handles batch c//2 and
16 query chunks of 128 rows.  Causal load-balance with a uniform program:
slot j (j=0..15) processes nkt_j = 2j+2 K-tiles (128 rows each); the core
with parity p takes q-chunk 2j+p (rows 128*(2j+p)..+128).  Parity 0 needs
2j+1 tiles (the extra one is masked to zero), parity 1 needs exactly 2j+2.
Per-core differences (batch data, q-row selection, causal masks) enter via
input data only, so all 8 cores run the same NEFF.

scores = Q K^T = emb_q (Wq^T Wk) emb_k^T, so K is never materialized:
M = Wq^T Wk once, qt = (emb_q M)^T, scores tile = embT_tile^T @ qt_tile.
V is projected straight into SBUF (no DRAM round trip) and stays resident.

Host-side prep (layout only, no model math): transpose + bf16-cast of inputs,
q-row gather, mask table construction.  All projections, scores, softmax and
AV run on device (bf16 matmuls, f32 accumulation/softmax).
"""

import math
import os
import sys
import types

import numpy as np
import ml_dtypes

B, S, D = 4, 4096, 1024
NCORES = 8
NSLOT = 16
CHUNK = 128          # q rows per slot
NKT = [2 * j + 2 for j in range(NSLOT)]   # K-tiles (128 rows) per slot
INV_SQRT_D = 1.0 / math.sqrt(D)
BF16 = ml_dtypes.bfloat16


# ---------------------------------------------------------------------------
# Environment patches (compiler workarounds + profiling hook shim)
# ---------------------------------------------------------------------------

def _install_patches():
    import json as _json
    import concourse.bass as bass

    if not getattr(bass.Bass, "_mw_patched", False):
        _orig_to_json = bass.Bass.to_json_bytes

        def to_json_bytes(self):
            # This walrus build rejects any instruction carrying more than one
            # sync wait ("Too many sync wait commands").  Split extra waits
            # onto single-wait NoOps inserted just before the instruction (the
            # engine executes them in order, so semantics are unchanged).
            raw = _orig_to_json(self)
            m = _json.loads(raw)
            ctr = 0
            changed = False
            for fn in m.get("functions", []):
                for bb in fn.get("blocks", []):
                    out = []
                    for inst in bb.get("instructions", []):
                        si = inst.get("sync_info")
                        if si:
                            waits = si.get("on_wait") or []
                            if len(waits) > 1:
                                changed = True
                                for w in waits[:-1]:
                                    ctr += 1
                                    out.append({
                                        "debug": inst.get("debug", 0),
                                        "engine": inst["engine"],
                                        "ins": [],
                                        "outs": [],
                                        "name": f"I-mw{ctr}",
                                        "opcode": "NoOp",
                                        "text_hint": "mwsplit",
                                        "sync_info": {"on_wait": [w],
                                                      "on_update": []},
                                    })
                                si["on_wait"] = [waits[-1]]
                        out.append(inst)
                    bb["instructions"] = out
            if not changed:
                return raw
            return _json.dumps(m).encode()

        bass.Bass.to_json_bytes = to_json_bytes
        bass.Bass._mw_patched = True

    # Don't upload NEFF/trace artifacts anywhere; keep them local.
    import concourse.bass_utils as bu
    bu.upload_artifacts = lambda tmpdir: tmpdir


def _install_ntff_hook() -> bool:
    """Register the axon NTFF profiling hook (missing module in this image)."""
    try:
        import antenv.axon_hooks  # noqa: F401
        return True
    except ImportError:
        pass
    try:
        mod = types.ModuleType("antenv.axon_hooks")
        state = {"hook": None}
        mod.set_axon_ntff_profile_hook = lambda h: state.__setitem__("hook", h)
        mod.get_axon_ntff_profile_hook = lambda: state["hook"]
        sys.modules["antenv.axon_hooks"] = mod
        import antenv
        antenv.axon_hooks = mod
        from trn_agent_boot.trn_boot import _ntff_profile_via_ctypes
        mod.set_axon_ntff_profile_hook(
            _ntff_profile_via_ctypes("/opt/axon/libaxon_pjrt.so"))
        return True
    except Exception:
        return False


# ---------------------------------------------------------------------------
# Graph
# ---------------------------------------------------------------------------

def _build_graph():
    import concourse.bass as bass
    import concourse.mybir as mybir
    import concourse.tile as tile

    f32 = mybir.dt.float32
    bf16 = mybir.dt.bfloat16
    Exp = mybir.ActivationFunctionType.Exp
    Copy = mybir.ActivationFunctionType.Copy

    nc = bass.Bass("TRN2", debug=False, num_devices=NCORES)

    embT_in = nc.dram_tensor("embT", [4, D, S // 4], bf16,
                         kind="ExternalInput")
    embqT_in = nc.dram_tensor("embqT", [4, D, 512], bf16,
                              kind="ExternalInput")
    wqs_in = nc.dram_tensor("wqs", [D, 128], bf16, kind="ExternalInput")
    wk_in = nc.dram_tensor("wkn", [D, D], bf16, kind="ExternalInput")
    wvT_in = nc.dram_tensor("wvT", [2, D, 512], bf16, kind="ExternalInput")
    masks_in = nc.dram_tensor("masks", [2, 128, CHUNK], bf16,
                              kind="ExternalInput")
    out_d = nc.dram_tensor("out", [NSLOT * CHUNK, D], bf16,
                           kind="ExternalOutput")

    with tile.TileContext(nc) as tc:
        with (
            tc.tile_pool(name="dram", bufs=1, space="DRAM") as dram,
            tc.tile_pool(name="wsb", bufs=1) as wsb,          # 2KB/part tiles
            tc.tile_pool(name="ktsb", bufs=1) as ktsb,        # embT resident
            tc.tile_pool(name="embs", bufs=8) as embs,        # embqT stream
            tc.tile_pool(name="wts", bufs=3) as wts,          # exp weights
            tc.tile_pool(name="outs", bufs=2) as outs,        # output stage
            tc.tile_pool(name="smalls", bufs=2) as smalls,
            tc.tile_pool(name="pmm", bufs=2, space="PSUM") as pmm,
            tc.tile_pool(name="ps", bufs=2, space="PSUM") as ps_pool,
            tc.tile_pool(name="pl", bufs=1, space="PSUM") as pl_pool,
        ):
            # constants
            ones = smalls.tile([128, 1], bf16, name="ones", tag="ones")
            nc.gpsimd.memset(ones[:], 1.0)
            mask_sb = []
            for t in range(2):
                mt = smalls.tile([128, CHUNK], bf16, name=f"mk{t}",
                                 tag=f"mk{t}")
                nc.scalar.dma_start(mt[:], masks_in[t, :, :])
                mask_sb.append(mt)

            # resident weights.  DMA issue order = arrival order: wqs+wv
            # first along with embT quarter 0 so Vproj starts early; wk (for
            # the M slice) after; embqT (for PP) last.
            wqs_sb, wk_n, wv_t = [], [], []
            for dc in range(8):
                t = wsb.tile([128, 128], bf16, name=f"wqs{dc}",
                             tag=f"wqs{dc}")
                nc.scalar.dma_start(t[:], wqs_in[dc * 128:(dc + 1) * 128, :])
                wqs_sb.append(t)
            for dc in range(8):
                t = wsb.tile([128, D], bf16, name=f"wv{dc}", tag=f"wv{dc}")
                wv_t.append(t)
            for eb in range(2):
                for dc in range(8):
                    nc.scalar.dma_start(
                        wv_t[dc][:, eb * 512:(eb + 1) * 512],
                        wvT_in[eb, dc * 128:(dc + 1) * 128, :])

            # resident emb^T tiles [128d, S], DMA'd in column quarters so
            # the first V tiles can start after ~1/4 of the data arrives.
            embt_sb = [ktsb.tile([128, S], bf16, name=f"et{dc}",
                                 tag=f"et{dc}") for dc in range(8)]
            for q in range(4):
                if q == 1:
                    for dc in range(8):
                        t = wsb.tile([128, D], bf16, name=f"wk{dc}",
                                     tag=f"wk{dc}")
                        nc.gpsimd.dma_start(
                            t[:], wk_in[dc * 128:(dc + 1) * 128, :])
                        wk_n.append(t)
                for dc in range(8):
                    eng = nc.gpsimd if (q == 0 and dc >= 4) else nc.sync
                    eng.dma_start(
                        embt_sb[dc][:, q * 1024:(q + 1) * 1024],
                        embT_in[q, dc * 128:(dc + 1) * 128, :])

            # ---------------- V projection (straight into SBUF) -----------
            # V tile st = rows [128*st, 128*st+128) of V = emb @ Wv^T, bf16.
            # Tiles 8..15 reuse the wk SBUF buffers (dead after the M slice).
            v_sb = []
            for st in range(32):
                if 8 <= st < 16:
                    vtag = f"wk{st - 8}"
                else:
                    vtag = f"v{st}"
                vt = wsb.tile([128, 1024], bf16, name=f"v{st}", tag=vtag)
                v_sb.append(vt)

            def vproj(st):
                psum = pmm.tile([128, 1024], f32, name=f"pv{st}", tag="mm")
                col = st * 128
                for eb in range(2):
                    for dc in range(8):
                        nc.tensor.matmul(
                            psum[:, eb * 512:(eb + 1) * 512],
                            embt_sb[dc][:, col:col + 128],
                            wv_t[dc][:, eb * 512:(eb + 1) * 512],
                            start=(dc == 0), stop=(dc == 7))
                nc.scalar.copy(v_sb[st][:], psum[:])

            for st in range(4):
                vproj(st)

            # ---------------- M = Wq^T @ Wk  [d_a, d_b] ----------------
            # scores = Q K^T = (emb_q Wq^T)(emb_k Wk^T)^T = emb_q M emb_k^T,
            # so the K projection never needs to be materialized.  Core c
            # computes only M rows [128c, 128c+128) (via its wqs column
            # slice of Wq) and the slices are AllGathered.
            m_loc = dram.tile([128, D], bf16, name="m_loc")
            m_ag = dram.tile([D, D], bf16, name="m_ag")
            psum = pmm.tile([128, 1024], f32, name="pm", tag="mm")
            for bb in range(2):
                for ec in range(8):
                    nc.tensor.matmul(
                        psum[:, bb * 512:(bb + 1) * 512],
                        wqs_sb[ec][:],
                        wk_n[ec][:, bb * 512:(bb + 1) * 512],
                        start=(ec == 0), stop=(ec == 7))
            for eb in range(2):
                mstage = outs.tile([128, 512], bf16, name=f"mst{eb}",
                                   tag="outs")
                nc.scalar.copy(mstage[:], psum[:, eb * 512:(eb + 1) * 512])
                nc.gpsimd.dma_start(m_loc[:, eb * 512:(eb + 1) * 512],
                                    mstage[:])
            nc.gpsimd.collective_compute(
                "AllGather",
                mybir.AluOpType.bypass,
                replica_groups=[list(range(NCORES))],
                ins=[m_loc.opt()],
                outs=[m_ag.opt()],
            )
            m_sb = []
            for ac in range(8):
                t = wsb.tile([128, D], bf16, name=f"m{ac}", tag=f"m{ac}")
                nc.gpsimd.dma_start(t[:], m_ag[ac * 128:(ac + 1) * 128, :])
                m_sb.append(t)

            for st in range(4, 32):
                vproj(st)

            # ------------- qt = (emb_q M)^T, kept in SBUF -------------
            qt_sb = [wsb.tile([128, D], bf16, name=f"qt{i}", tag=f"qt{i}")
                     for i in range(16)]
            for qb in range(4):
                eblk = []
                for dc in range(8):
                    t = embs.tile([128, 512], bf16, name=f"eqT{qb}_{dc}",
                                  tag="embs")
                    nc.scalar.dma_start(
                        t[:], embqT_in[qb, dc * 128:(dc + 1) * 128, :])
                    eblk.append(t)
                for bc in range(8):
                    psum = pmm.tile([128, 512], f32, name=f"pq{qb}_{bc}",
                                    tag="mm")
                    for ac in range(8):
                        nc.tensor.matmul(
                            psum[:], m_sb[ac][:, bc * 128:(bc + 1) * 128],
                            eblk[ac][:], start=(ac == 0), stop=(ac == 7))
                    half = qb // 2
                    off = (qb % 2) * 512
                    nc.scalar.copy(qt_sb[bc * 2 + half][:, off:off + 512],
                                   psum[:])

            # ---------------- attention ----------------
            # Emission order alternates long and short slots so a short
            # slot's boundary latency (recip/scale/out) hides under the
            # following long slot's score stream.
            order = []
            for i in range(NSLOT // 2):
                order += [NSLOT - 1 - i, i]
            for e, j in enumerate(order):
                nkt = NKT[j]
                h, c0 = j // 8, (j % 8) * CHUNK
                qt_tiles = [qt_sb[ec * 2 + h][:, c0:c0 + CHUNK]
                            for ec in range(8)]

                l_ps = pl_pool.tile([128, 1], f32, name=f"l{j}",
                                    tag=f"l{e % 2}")
                av = pmm.tile([128, 1024], f32, name=f"av{j}", tag="mm")

                for kt in range(nkt):
                    s_ps = ps_pool.tile([128, CHUNK], f32, name=f"s{j}_{kt}",
                                        tag="s")
                    for dc in range(8):
                        nc.tensor.matmul(
                            s_ps[:], embt_sb[dc][:, kt * 128:(kt + 1) * 128],
                            qt_tiles[dc], start=(dc == 0), stop=(dc == 7))

                    wt = wts.tile([128, CHUNK], bf16, name=f"w{j}_{kt}",
                                  tag="wts")
                    nc.scalar.activation(wt[:], s_ps[:], Exp, bias=0.0,
                                         scale=INV_SQRT_D)
                    if kt >= nkt - 2:
                        nc.vector.tensor_mul(wt[:], wt[:],
                                             mask_sb[kt - (nkt - 2)][:])

                    first, last = kt == 0, kt == nkt - 1
                    for eb in range(2):
                        nc.tensor.matmul(
                            av[:, eb * 512:(eb + 1) * 512], wt[:],
                            v_sb[kt][:, eb * 512:(eb + 1) * 512],
                            start=first, stop=last)
                    # l rowsum shares wt as the stationary operand; issued
                    # last so its weight load prefetches under the AV streams.
                    nc.tensor.matmul(l_ps[:], wt[:], ones[:],
                                     start=first, stop=last)

                r_sb = smalls.tile([128, 1], f32, name=f"r{j}", tag="r")
                nc.vector.reciprocal(r_sb[:], l_ps[:])
                row = j * CHUNK
                for eb in range(2):
                    o_sb = outs.tile([128, 512], bf16, name=f"o{j}_{eb}",
                                     tag="outs")
                    if eb == 0:
                        nc.scalar.activation(
                            o_sb[:], av[:, 0:512], Copy, bias=0.0,
                            scale=r_sb[:])
                    else:
                        nc.vector.tensor_scalar_mul(
                            o_sb[:], av[:, 512:1024], r_sb[:])
                    nc.sync.dma_start(
                        out_d[row:row + CHUNK, eb * 512:(eb + 1) * 512],
                        o_sb[:])

    return nc


_CACHED = {}


def _get_graph():
    if "nc" not in _CACHED:
        _install_patches()
        _CACHED["nc"] = _build_graph()
    return _CACHED["nc"]


# ---------------------------------------------------------------------------
# Host-side staging
# ---------------------------------------------------------------------------

def _chunks(parity):
    return [2 * j + parity for j in range(NSLOT)]


def _masks(parity):
    # mask tile t (t=0,1) applies to K-tile (2j+t) of slot j:
    # keep iff 128*parity + x >= 128*t + r  (r = k row in partition dim,
    # x = q col in free dim).
    m = np.zeros((2, 128, CHUNK), dtype=np.float32)
    r = np.arange(128)[:, None]
    x = np.arange(CHUNK)[None, :]
    for t in range(2):
        m[t] = ((128 * parity + x) >= (128 * t + r)).astype(np.float32)
    return m.astype(BF16)


def kernel(embeddings, Wq, Wk, Wv):
    embeddings = np.asarray(embeddings, dtype=np.float32)
    Wq = np.asarray(Wq, dtype=np.float32)
    Wk = np.asarray(Wk, dtype=np.float32)
    Wv = np.asarray(Wv, dtype=np.float32)

    nc = _get_graph()
    from concourse.bass_utils import run_bass_kernel_spmd

    wkn = Wk.astype(BF16)
    wvT = np.ascontiguousarray(
        Wv.T.reshape(D, 2, 512).transpose(1, 0, 2)).astype(BF16)
    masks_by_par = [_masks(0), _masks(1)]

    in_maps = []
    for c in range(NCORES):
        b, par = divmod(c, 2)
        emb_b = embeddings[b]
        embT = np.ascontiguousarray(
            emb_b.T.reshape(D, 4, S // 4).transpose(1, 0, 2)).astype(BF16)
        rows = np.concatenate(
            [np.arange(g * CHUNK, (g + 1) * CHUNK) for g in _chunks(par)])
        embq = emb_b[rows]
        embqT = np.ascontiguousarray(
            embq.T.reshape(D, 4, 512).transpose(1, 0, 2)).astype(BF16)
        in_maps.append({
            "embT": embT,
            "embqT": embqT,
            "wqs": np.ascontiguousarray(
                Wq[:, c * 128:(c + 1) * 128]).astype(BF16),
            "wkn": wkn,
            "wvT": wvT,
            "masks": masks_by_par[par],
        })

    trace = bool(int(os.environ.get("BASS_KERNEL_TRACE", "0")))
    kwargs = {}
    if trace:
        kwargs["trace"] = _install_ntff_hook()

    res = run_bass_kernel_spmd(nc, in_maps, core_ids=list(range(NCORES)),
                               **kwargs)
    _CACHED["last_result"] = res

    out = np.empty((B, S, D), dtype=np.float32)
    for c in range(NCORES):
        b, par = divmod(c, 2)
        core_out = res.results[c]["out"].astype(np.float32)
        for j, g in enumerate(_chunks(par)):
            out[b, g * CHUNK:(g + 1) * CHUNK] = \
                core_out[j * CHUNK:(j + 1) * CHUNK]
    return out
